# revision 1
# baseline (speedup 1.0000x reference)
"""Cross-attention layer on 8 TRN2 NeuronCores.

Sharding: core i -> (batch b = i//2, head-group g = i%2); each core computes
its head-group's contribution to out[b] through Wo; the host sums the two
partial products per batch (row-split of Wo => partial-sum reduction).

Device kernel works in transposed layout ([channels, tokens]) so the softmax
reduction is along the matmul free axis:
  Q^T = Wq_g^T x^T, K^T = Wk_g^T ctx^T, V = ctx Wv_g (+ ones column)
  scores^T_h = K_h Q_h^T  (contraction over head_dim=64)
  E = exp(scores^T/32) * mask^T      (no max subtraction; |scores/32| ~ 1.5)
  U = V'^T E  (per s-tile accumulation; row 64 = softmax denominator)
  O^T = U[0:64] * exp(-ln(U[64]))    (reciprocal via ACT ln/exp; then
                                      gpsimd partition-broadcast + DVE mult)
  out_partial = O^T^T Wo_g           (host adds core pairs)

Dtype split: x/ctx/Wq/Wk/Wv, Q^T/K^T/V', probs run in bf16 (pre-softmax
noise is negligible, probs/V noise ~0.5%); U accumulates in fp32 PSUM; the
normalized O^T and the Wo projection run in float32r (~1e-4).
"""

import os
import numpy as np
import ml_dtypes

import concourse.mybir as mybir
from concourse import bacc
import concourse.tile as tile
from concourse.bass_utils import run_bass_kernel_spmd

B, T, TC = 4, 1024, 1024
C, CTX_C, H = 1024, 1024, 16
HD = C // H            # 64
P = 128
NCORES = 8
HG = 2                 # head groups
HPG = H // HG          # 8 heads per core
CG = HPG * HD          # 512 channels per group
NT = 512               # matmul free-dim chunk
KO = C // P            # 8 contraction tiles for projections
MQ = CG // P           # 4 partition-tiles of Q^T/K^T
SO = TC // P           # 8 s-tiles
T2 = T // NT           # 2 t-chunks
KP = CG // P           # 4 contraction tiles for the out projection
F32 = mybir.dt.float32
F32R = mybir.dt.float32r
BF16 = mybir.dt.bfloat16
ALU = mybir.AluOpType
ACTF = mybir.ActivationFunctionType

_CACHED_NC = None


def _ensure_ntff_hook():
    """Register the axon NTFF profiling hook if the image's antenv lacks it."""
    try:
        from antenv.axon_hooks import get_axon_ntff_profile_hook  # noqa: F401
        return
    except ImportError:
        pass
    import sys
    import types
    try:
        from trn_agent_boot.trn_boot import _ntff_profile_via_ctypes
        hook = _ntff_profile_via_ctypes("/opt/axon/libaxon_pjrt.so")
    except Exception:
        hook = None
    mod = types.ModuleType("antenv.axon_hooks")
    mod.get_axon_ntff_profile_hook = lambda: hook
    mod.set_axon_ntff_profile_hook = lambda h: None
    sys.modules["antenv.axon_hooks"] = mod
    import antenv
    antenv.axon_hooks = mod


def _hp(h):
    """Partition slice of local head h inside a [128, MQ, ...] channel tile."""
    lo = (h % 2) * HD
    return slice(lo, lo + HD)


def _build_program():
    nc = bacc.Bacc("TRN2", target_bir_lowering=False, debug=False,
                   num_devices=NCORES)
    xT = nc.dram_tensor("xT", [C, T], BF16, kind="ExternalInput").ap()
    ctxT = nc.dram_tensor("ctxT", [CTX_C, TC], BF16, kind="ExternalInput").ap()
    maskT = nc.dram_tensor("maskT", [TC, T], BF16, kind="ExternalInput").ap()
    wq = nc.dram_tensor("wq", [C, CG], BF16, kind="ExternalInput").ap()
    wk = nc.dram_tensor("wk", [CTX_C, CG], BF16, kind="ExternalInput").ap()
    wv = nc.dram_tensor("wv", [CTX_C, CG], BF16, kind="ExternalInput").ap()
    wo = nc.dram_tensor("wo", [CG, C], F32, kind="ExternalInput").ap()
    out = nc.dram_tensor("out", [T, C], F32, kind="ExternalOutput").ap()

    with tile.TileContext(nc) as tc:
        with (
            tc.tile_pool(name="persist", bufs=1) as persist,
            tc.tile_pool(name="work", bufs=3) as work,
            tc.tile_pool(name="psmm", bufs=2, space="PSUM") as psmm,
            tc.tile_pool(name="pssc", bufs=2, space="PSUM") as pssc,
            tc.tile_pool(name="psu", bufs=2, space="PSUM") as psu_pool,
        ):
            qt_sb = persist.tile([P, MQ, T], BF16)            # Q^T [(h,d), t]
            kt_sb = persist.tile([P, MQ, TC], BF16)           # K^T [(h,d), s]
            vp_sb = persist.tile([P, SO, HPG, HD + 1], BF16)  # V' + ones col
            mask_sb = persist.tile([P, SO, T], BF16)          # mask^T
            ot_sb = persist.tile([P, KP, T], F32R)            # O^T normalized
            wo_sb = persist.tile([P, KP, C], F32R)
            xT_sb = persist.tile([P, KO, T], BF16)
            ctxT_sb = persist.tile([P, KO, TC], BF16)
            wq_sb = persist.tile([P, KO, CG], BF16)
            wk_sb = persist.tile([P, KO, CG], BF16)
            wv_sb = persist.tile([P, KO, CG], BF16)

            nc.gpsimd.memset(vp_sb[:, :, :, HD:HD + 1], 1.0)

            xT_r = xT.rearrange("(ko p) t -> p ko t", p=P)
            ctxT_r = ctxT.rearrange("(ko p) t -> p ko t", p=P)
            nc.sync.dma_start(wq_sb, wq.rearrange("(ko p) m -> p ko m", p=P))
            for kc in range(KO):   # chunked so stage A starts early
                nc.sync.dma_start(xT_sb[:, kc], xT_r[:, kc])
            nc.sync.dma_start(wk_sb, wk.rearrange("(ko p) m -> p ko m", p=P))
            nc.sync.dma_start(wv_sb, wv.rearrange("(ko p) m -> p ko m", p=P))
            for kc in range(KO):
                nc.sync.dma_start(ctxT_sb[:, kc], ctxT_r[:, kc])
            nc.sync.dma_start(mask_sb,
                              maskT.rearrange("(so p) t -> p so t", p=P))
            nc.sync.dma_start(
                wo_sb, wo.rearrange("(ko p) n -> p ko n", p=P).bitcast(F32R))

            # ---- Stage A/B: projections Q^T, K^T, V ----
            for m in range(MQ):          # Q^T = Wq^T x^T
                for t2 in range(T2):
                    ps = psmm.tile([P, NT], F32, tag="mm512")
                    for kc in range(KO):
                        nc.tensor.matmul(
                            ps, wq_sb[:, kc, m * P:(m + 1) * P],
                            xT_sb[:, kc, t2 * NT:(t2 + 1) * NT],
                            start=(kc == 0), stop=(kc == KO - 1))
                    nc.vector.tensor_copy(
                        qt_sb[:, m, t2 * NT:(t2 + 1) * NT], ps)
            for m in range(MQ):          # K^T = Wk^T ctx^T
                for s2 in range(T2):
                    ps = psmm.tile([P, NT], F32, tag="mm512")
                    for kc in range(KO):
                        nc.tensor.matmul(
                            ps, wk_sb[:, kc, m * P:(m + 1) * P],
                            ctxT_sb[:, kc, s2 * NT:(s2 + 1) * NT],
                            start=(kc == 0), stop=(kc == KO - 1))
                    nc.vector.tensor_copy(
                        kt_sb[:, m, s2 * NT:(s2 + 1) * NT], ps)
            for so in range(SO):         # V = ctx Wv  (natural layout)
                ps = psmm.tile([P, NT], F32, tag="mm512")
                for kc in range(KO):
                    nc.tensor.matmul(
                        ps, ctxT_sb[:, kc, so * P:(so + 1) * P],
                        wv_sb[:, kc, :],
                        start=(kc == 0), stop=(kc == KO - 1))
                nc.vector.tensor_copy(
                    vp_sb[:, so, :, 0:HD],
                    ps.rearrange("p (h d) -> p h d", h=HPG))

            # ---- Stage C: attention per (head, t-chunk), software-pipelined ----
            units = [(h, t2) for h in range(HPG) for t2 in range(T2)]

            def scores_unit(u):
                h, t2 = units[u]
                et = work.tile([P, SO, NT], BF16, tag="exp")
                for j in range(SO // 2):   # s-tile pairs share a 2-bank psum
                    ps = pssc.tile([P, 2 * NT], F32, tag="ps_sc")
                    for i in range(2):
                        so = 2 * j + i
                        nc.tensor.matmul(
                            ps[:, i * NT:(i + 1) * NT],
                            kt_sb[_hp(h), h // 2, so * P:(so + 1) * P],
                            qt_sb[_hp(h), h // 2, t2 * NT:(t2 + 1) * NT],
                            start=True, stop=True)
                    nc.scalar.activation(
                        et[:, 2 * j:2 * j + 2, :].rearrange("p a b -> p (a b)"),
                        ps, ACTF.Exp, scale=1.0 / 32.0)
                    nc.vector.tensor_tensor(
                        et[:, 2 * j:2 * j + 2, :],
                        et[:, 2 * j:2 * j + 2, :],
                        mask_sb[:, 2 * j:2 * j + 2, t2 * NT:(t2 + 1) * NT],
                        ALU.mult)
                return et

            def pv_unit(u, et):
                h, t2 = units[u]
                psu = psu_pool.tile([HD + 1, NT], F32, tag="ps_u")
                for so in range(SO):
                    nc.tensor.matmul(
                        psu, vp_sb[:, so, h, :], et[:, so, :],
                        start=(so == 0), stop=(so == SO - 1))
                # reciprocal via exp(-ln(den)) on ACT: keeps DVE free for the
                # mask mults that gate PV; costs one Exp<->Ln table swap pair
                ln = work.tile([1, NT], F32, tag="ln")
                nc.scalar.activation(ln, psu[HD:HD + 1, :], ACTF.Ln)
                rc = work.tile([1, NT], F32, tag="recip")
                nc.scalar.activation(rc, ln, ACTF.Exp, scale=-1.0)
                bc = work.tile([HD, NT], F32, tag="bcast")
                nc.gpsimd.partition_broadcast(bc, rc)
                nc.vector.tensor_tensor(
                    ot_sb[_hp(h), h // 2, t2 * NT:(t2 + 1) * NT],
                    psu[0:HD, :], bc, ALU.mult)

            # run scores two units ahead of PV so ACT/DVE hiccups
            # (e.g. the per-unit Exp<->Ln table swap) never starve the PE
            pending = [scores_unit(0), scores_unit(1)]
            for u in range(len(units)):
                if u + 2 < len(units):
                    pending.append(scores_unit(u + 2))
                pv_unit(u, pending.pop(0))

            # ---- Stage D: out_partial = O Wo ----
            for tm in range(T // P):
                for c2 in range(C // NT):
                    ps = psmm.tile([P, NT], F32, tag="mm512")
                    for kp in range(KP):
                        nc.tensor.matmul(
                            ps, ot_sb[:, kp, tm * P:(tm + 1) * P],
                            wo_sb[:, kp, c2 * NT:(c2 + 1) * NT],
                            start=(kp == 0), stop=(kp == KP - 1))
                    o_sb = work.tile([P, NT], F32, tag="out")
                    nc.scalar.activation(o_sb, ps, ACTF.Copy)
                    nc.sync.dma_start(
                        out[tm * P:(tm + 1) * P, c2 * NT:(c2 + 1) * NT],
                        o_sb)
    nc.compile()
    return nc


def _get_program():
    global _CACHED_NC
    if _CACHED_NC is None:
        _CACHED_NC = _build_program()
    return _CACHED_NC


def kernel(x, context, attn_mask, Wq, Wk, Wv, Wo):
    x = np.asarray(x, dtype=np.float32)
    context = np.asarray(context, dtype=np.float32)
    attn_mask = np.asarray(attn_mask)
    Wq = np.asarray(Wq, dtype=np.float32)
    Wk = np.asarray(Wk, dtype=np.float32)
    Wv = np.asarray(Wv, dtype=np.float32)
    Wo = np.asarray(Wo, dtype=np.float32)

    nc = _get_program()
    bf = ml_dtypes.bfloat16
    in_maps = []
    for i in range(NCORES):
        b, g = i // 2, i % 2
        cs = slice(g * CG, (g + 1) * CG)
        in_maps.append({
            "xT": np.ascontiguousarray(x[b].T).astype(bf),
            "ctxT": np.ascontiguousarray(context[b].T).astype(bf),
            "maskT": np.ascontiguousarray(attn_mask[b, 0].T).astype(bf),
            "wq": np.ascontiguousarray(Wq[:, cs]).astype(bf),
            "wk": np.ascontiguousarray(Wk[:, cs]).astype(bf),
            "wv": np.ascontiguousarray(Wv[:, cs]).astype(bf),
            "wo": np.ascontiguousarray(Wo[cs, :]),
        })

    profile = os.environ.get("KERNEL_PROFILE", "0") == "1"
    if profile:
        _ensure_ntff_hook()
    res = run_bass_kernel_spmd(
        nc, in_maps, list(range(NCORES)),
        trace=profile, trace_cores=[0] if profile else None)
    if profile:
        kernel.last_exec_time_ns = res.exec_time_ns
        kernel.last_trace = res.instructions_and_trace

    out = np.empty((B, T, C), dtype=np.float32)
    for b in range(B):
        out[b] = res.results[2 * b]["out"] + res.results[2 * b + 1]["out"]
    return out

